# revision 28
# baseline (speedup 1.0000x reference)
"""Trainium2 Bass kernel for nn_CapsuleLayerTSV (capsule routing over 40 adapters).

Strategy (8 NeuronCores, two SPMD NEFFs, no collectives):
  Phase 1 (expert-parallel): allowed adapters (tsv[t] != 0) sharded across
    cores, ka=3 slots each (pad slots zero-filled). Priors computed as
    float32r matmuls (12-bit-mantissa PE mode, 1 cyc/col) — [256,600]@[600,600]
    per adapter with capsules folded into the free dim.
  Host: reassemble priors, re-shard by the output's flat row space; ship each
    phase-2 core BOTH layouts of its priors (k-major for agreements, d-major
    for votes) so no on-chip transpose is needed.
  Phase 2 (pair-parallel): 3-iteration dynamic routing for 96 (n,b) pairs per
    core using big fused DVE ops (broadcast-multiply + 3D tensor_reduce over
    the innermost axis) instead of per-adapter op chains. Projection
    u[6400,3] @ lwg[3,768] in float32r; output staged to SBUF as bf16 and
    written as a bf16 DRAM tensor (host upcasts to f32).
"""

import sys

sys.path.insert(0, "/opt/trn_rl_repo")

import numpy as np
import ml_dtypes

import concourse.bass as bass
import concourse.mybir as mybir
import concourse.tile as tile
from concourse.bass import broadcast_tensor_aps
from concourse.bass_utils import run_bass_kernel_spmd

F32 = mybir.dt.float32
F32R = mybir.dt.float32r
BF16 = mybir.dt.bfloat16
F16 = mybir.dt.float16
AX = mybir.AxisListType
ALU = mybir.AluOpType
ACTF = mybir.ActivationFunctionType

NC = 8
B = 256
ADP = 40
CAPS = 3
INCH = 600
D = 200
M = 768
ND = CAPS * D  # 600
PP = CAPS * B // NC  # 96 (n,b) pairs per core in phase 2
ROWS_PER_CORE = PP * D // CAPS  # 6400 output rows per core
JCH = ROWS_PER_CORE // 128  # 50 row-chunks
KC = 5  # phase-1 contraction chunks of 120
KCS = INCH // KC  # 120

_BUILD_CACHE = {}
USE_GPSIMD_SPLIT = True


def _split_multiwait_waits(nc):
    """walrus caps sync-waits at ONE per instruction. For instructions executed
    by an in-order engine sequencer (everything except queue-executed DMAs),
    splitting the wait list across preceding 1-wait NoOps/Drains on the same
    engine is semantics-preserving."""
    for fn in nc.m.functions:
        for blk in fn.blocks:
            out = []
            for inst in blk.instructions:
                si = getattr(inst, "sync_info", None)
                if (
                    si is not None
                    and si.on_wait
                    and len(si.on_wait) > 1
                    and not isinstance(inst, mybir.InstDMACopy)
                    and getattr(inst, "engine", None) is not None
                ):
                    waits = list(si.on_wait)
                    cls = (
                        mybir.InstDrain
                        if isinstance(inst, mybir.InstDrain)
                        else mybir.InstNoOp
                    )
                    for i, w in enumerate(waits[:-1]):
                        extra = cls(
                            name=f"{inst.name}_w{i}",
                            engine=inst.engine,
                            sync_info=mybir.SyncInfo(on_wait=[w], on_update=[]),
                            bass_nofuse=True,
                        )
                        nc.register_instruction(extra)
                        out.append(extra)
                    si.on_wait = waits[-1:]
                out.append(inst)
            blk.instructions = out


# test/debug hook: kernel() appends the BassKernelResults of each phase here
LAST_RESULTS = []


def _build_phase1(ka):
    """SPMD program: priors for `ka` adapter slots per core, float32r.

    inputs : xw  [ka, 600, 856] f32   (cols 0:256 = x^T slice, 256:856 = W [c, n*d])
    output : pri [ka, 2, 128, 600] f32  (priors [b, n*d], b in 2 chunks of 128)
    """
    nc = bass.Bass()
    xw = nc.declare_dram_parameter("xw", [ka, INCH, B + ND], F32R, isOutput=False)
    pri = nc.declare_dram_parameter("pri", [ka, 2, 128, ND], F32, isOutput=True)

    with tile.TileContext(nc) as tc:
        with (
            tc.tile_pool(name="xt", bufs=3) as xt_pool,
            tc.tile_pool(name="ob", bufs=4) as ob_pool,
            tc.tile_pool(name="ps", bufs=2, space="PSUM") as ps_pool,
        ):
            for k in range(ka):
                # per-chunk HWDGE DMAs so matmuls start after the first chunk
                xwc = []
                for ci in range(KC):
                    cchunk = xt_pool.tile(
                        [KCS, B + ND], F32R, tag=f"xw{ci}", name=f"xw{k}_{ci}"
                    )
                    nc.sync.dma_start(
                        out=cchunk[:, :], in_=xw[k, ci * KCS:(ci + 1) * KCS, :]
                    )
                    xwc.append(cchunk)
                for bc in range(2):
                    for nh in range(2):
                        ps = ps_pool.tile([128, ND // 2], F32, tag=f"ps{bc}{nh}")
                        for ci in range(KC):
                            nc.tensor.matmul(
                                ps[:, :],
                                xwc[ci][:, bc * 128:(bc + 1) * 128],
                                xwc[ci][:, B + nh * 300:B + (nh + 1) * 300],
                                start=(ci == 0),
                                stop=(ci == KC - 1),
                            )
                        osb = ob_pool.tile([128, ND // 2], F32, tag=f"o{bc}{nh}")
                        dst = pri[k, bc, :, nh * 300:(nh + 1) * 300]
                        # copy psum->SBUF on vector/scalar; a 1-elem gpsimd
                        # absorber pulls the copy's sem into gpsimd's clock so
                        # the store DMA carries only its queue-slot wait
                        # (walrus allows ONE wait per DMA).
                        if nh == 0:
                            nc.vector.tensor_copy(osb[:, :], ps[:, :])
                        else:
                            nc.scalar.copy(osb[:, :], ps[:, :])
                        pab = ob_pool.tile([1, 1], F32, tag=f"pab{bc}{nh}")
                        nc.gpsimd.tensor_copy(pab[:, :], osb[0:1, 0:1])
                        nc.gpsimd.dma_start(out=dst, in_=osb[:, :])
    return nc


def _build_phase2(A):
    """SPMD program: routing for 96 (n,b) pairs + bf16 projection per core.

    inputs : pk   [96, A*200] f32  (k-major priors: [pair, k, d])
             pd   [96, 200*A] f16  (d-major priors: [pair, d, k], vote path)
             o1in [96, 200] f32    (squash of the uniform vote, host-computed)
             tsvr [96, A] f32      (tsv[t] over allowed adapters, replicated)
             lwg  [3, 768] bf16    (gate-folded projection, larger_b == 0)
    output : outc [6400, 768] bf16

    Algebra: the squashed output o_i never materializes for i>=2 —
    agreements use the unsquashed-but-normalized vote v (products stay in
    fp16 range) and the squash factor f = sqrt(sq)/(1+sq) scales the
    21-element agreement row afterwards: aT = f * sum_d(Pk * v).
    """
    nc = bass.Bass()
    pk = nc.declare_dram_parameter("pk", [PP, A * D], F32, isOutput=False)
    o1in = nc.declare_dram_parameter("o1in", [PP, D], F32, isOutput=False)
    pd = nc.declare_dram_parameter("pd", [PP, D * A], F16, isOutput=False)
    tsvr = nc.declare_dram_parameter("tsvr", [PP, A], F32, isOutput=False)
    lwg = nc.declare_dram_parameter("lwg", [CAPS, M], BF16, isOutput=False)
    outc = nc.declare_dram_parameter("outc", [ROWS_PER_CORE, M], BF16, isOutput=True)
    uTd = nc.dram_tensor("uTd", [CAPS, ROWS_PER_CORE], BF16)  # u^T staging

    KH = (A // 2) * D  # k-split point of the pk load/first multiply
    uid = [0]

    with tile.TileContext(nc) as tc:
        with (
            tc.tile_pool(name="sb", bufs=1) as sb,
            tc.tile_pool(name="ob", bufs=4) as ob_pool,
            tc.tile_pool(name="ps", bufs=3, space="PSUM") as ps_pool,
        ):
            def fresh(shape, dtype=F32, pfx="t"):
                uid[0] += 1
                return sb.tile(
                    shape, dtype, tag=f"{pfx}{uid[0]}", name=f"{pfx}{uid[0]}"
                )

            # ---- loads (pk split in two so ag1 starts on the first half) ----
            Pk = sb.tile([PP, A * D], F32, tag="Pk")
            nc.sync.dma_start(out=Pk[:, :KH], in_=pk[:, :KH])
            nc.sync.dma_start(out=Pk[:, KH:], in_=pk[:, KH:])
            Pk3 = Pk[:, :].rearrange("p (k d) -> p k d", k=A)
            o1 = sb.tile([PP, D], F32, tag="o1")
            nc.sync.dma_start(out=o1[:, :], in_=o1in[:, :])
            Pd = sb.tile([PP, D * A], F16, tag="Pd")
            nc.sync.dma_start(out=Pd[:, :], in_=pd[:, :])
            Pd3 = Pd[:, :].rearrange("p (d k) -> p d k", d=D)
            tsv_t = sb.tile([PP, A], F32, tag="tsv")
            nc.sync.dma_start(out=tsv_t[:, :], in_=tsvr[:, :])
            lwg_t = sb.tile([CAPS, M], BF16, tag="lwg")
            nc.sync.dma_start(out=lwg_t[:, :], in_=lwg[:, :])

            scrK = sb.tile([PP, A * D], F16, tag="scrK")
            scrK3 = scrK[:, :].rearrange("p (k d) -> p k d", k=A)
            scrD = sb.tile([PP, D * A], F16, tag="scrD")
            scrD3 = scrD[:, :].rearrange("p (d k) -> p d k", d=D)

            psj = ps_pool.tile([128, 512], F32, tag="psj", bufs=1)

            def pe_warm():
                nc.tensor.matmul(
                    psj[:, :], lwg_t[:, 0:128], lwg_t[:, 0:512],
                    start=True, stop=True,
                )

            def agree_raw(v_t, aT, split=False):
                """aT[p,k] = sum_d Pk[p,k,d] * v_t[p,d]  (mult + X-reduce)."""
                v3 = v_t[:, :].rearrange("p (u d) -> p u d", u=1)
                v3b, Pk3b = broadcast_tensor_aps(v3, Pk3)
                if split:
                    kh = A // 2
                    nc.vector.tensor_tensor(
                        out=scrK3[:, :kh, :], in0=Pk3b[:, :kh, :],
                        in1=v3b[:, :kh, :], op=ALU.mult,
                    )
                    nc.vector.tensor_tensor(
                        out=scrK3[:, kh:, :], in0=Pk3b[:, kh:, :],
                        in1=v3b[:, kh:, :], op=ALU.mult,
                    )
                else:
                    nc.vector.tensor_tensor(
                        out=scrK3, in0=Pk3b, in1=v3b, op=ALU.mult
                    )
                nc.vector.tensor_reduce(aT[:, :], scrK3, AX.X, ALU.add)

            def vote_weighted(e_t, vs):
                """vs[p,d] = sum_k e_t[p,k] * Pd[p,d,k]  (all-fp16 mult + X-reduce)."""
                e3 = e_t[:, :].rearrange("p (u k) -> p u k", u=1)
                e3b, Pd3b = broadcast_tensor_aps(e3, Pd3)
                nc.vector.tensor_tensor(out=scrD3, in0=Pd3b, in1=e3b, op=ALU.mult)
                nc.vector.tensor_reduce(vs[:, :], scrD3, AX.X, ALU.add)

            def softmax_from_logit(logit):
                """returns (e fp16, dinv f32): e = exp(logit - max); dinv = 1/sum."""
                rmax = fresh([PP, 1], F32, "rmx")
                am = fresh([PP, A], F32, "am")
                e = fresh([PP, A], F16, "e")
                dsum = fresh([PP, 1], F32, "dsm")
                dinv = fresh([PP, 1], F32, "dnv")
                nc.vector.tensor_reduce(rmax[:, :], logit[:, :], AX.X, ALU.max)
                nc.vector.tensor_scalar(
                    out=am[:, :], in0=logit[:, :], scalar1=rmax[:, 0:1],
                    scalar2=None, op0=ALU.subtract,
                )
                nc.scalar.activation(
                    e[:, :], am[:, :], ACTF.Exp, accum_out=dsum[:, 0:1]
                )
                nc.vector.reciprocal(dinv[:, :], dsum[:, :])
                return e, dinv

            def squash_scale(v_t):
                """f = sqrt(sq)/(1+sq), sq = sum(v_t^2); sqrt via DVE pow."""
                junk = fresh([PP, D], F32, "sqj")
                sq = fresh([PP, 1], F32, "sq")
                sqs = fresh([PP, 1], F32, "sqs")
                sp1 = fresh([PP, 1], F32, "sp1")
                rec = fresh([PP, 1], F32, "rec")
                f = fresh([PP, 1], F32, "f")
                nc.vector.scalar_tensor_tensor(
                    out=junk[:, :], in0=v_t[:, :], scalar=1.0, in1=v_t[:, :],
                    op0=ALU.mult, op1=ALU.mult, accum_out=sq[:, 0:1],
                )
                nc.scalar.sqrt(sqs[:, :], sq[:, :])
                nc.vector.tensor_scalar(
                    out=sp1[:, :], in0=sq[:, :], scalar1=1.0, scalar2=None,
                    op0=ALU.add,
                )
                nc.vector.reciprocal(rec[:, :], sp1[:, :])
                nc.vector.tensor_tensor(
                    out=f[:, :], in0=sqs[:, :], in1=rec[:, :], op=ALU.mult
                )
                return f

            def iteration(e_t, dinv, lg_prev, last):
                """One routing iteration from softmax weights; returns next
                logits (or the final normalized vote when last=True)."""
                vs = fresh([PP, D], F32, "vs")
                vote_weighted(e_t, vs)
                v = fresh([PP, D], F32, "v")
                nc.vector.tensor_scalar(
                    out=v[:, :], in0=vs[:, :], scalar1=dinv[:, 0:1],
                    scalar2=None, op0=ALU.mult,
                )
                if last:
                    return v
                f = squash_scale(v)
                agp = fresh([PP, A], F32, "agp")
                agree_raw(v, agp)
                aT = fresh([PP, A], F32, "aT")
                nc.vector.tensor_scalar(
                    out=aT[:, :], in0=agp[:, :], scalar1=f[:, 0:1],
                    scalar2=None, op0=ALU.mult,
                )
                lg = fresh([PP, A], F32, "lg")
                nc.vector.scalar_tensor_tensor(
                    out=lg[:, :], in0=lg_prev[:, :], scalar=1.0, in1=aT[:, :],
                    op0=ALU.mult, op1=ALU.add,
                )
                nc.vector.tensor_tensor(
                    out=lg[:, :], in0=lg[:, :], in1=tsv_t[:, :], op=ALU.mult
                )
                return lg

            # ---- iteration 1: uniform probs; o1 shipped from host ----
            aT1 = fresh([PP, A], F32, "aT1")
            agree_raw(o1, aT1, split=True)
            lg1 = fresh([PP, A], F32, "lg1")
            nc.vector.tensor_tensor(
                out=lg1[:, :], in0=aT1[:, :], in1=tsv_t[:, :], op=ALU.mult
            )
            pe_warm()

            # ---- iteration 2 ----
            e2, dinv2 = softmax_from_logit(lg1)
            pe_warm()
            lg2 = iteration(e2, dinv2, lg1, last=False)
            pe_warm()

            # ---- iteration 3: final vote ----
            e3, dinv3 = softmax_from_logit(lg2)
            pe_warm()
            vs3 = fresh([PP, D], F32, "vs3")
            vote_weighted(e3, vs3)
            vb16 = fresh([PP, D], BF16, "vb16")
            nc.vector.tensor_scalar(
                out=vb16[:, :], in0=vs3[:, :], scalar1=dinv3[:, 0:1],
                scalar2=None, op0=ALU.mult,
            )
            pe_warm()

            # ---- deinterleave vote stream into u^T rows (via DRAM) ----
            vstack = fresh([PP // CAPS, CAPS * D], BF16, "vstk")
            nc.gpsimd.dma_start(
                out=vstack[:, :].rearrange("q (m d) -> q m d", m=CAPS),
                in_=vb16[:, :],
            )
            uT2 = fresh([PP // CAPS, CAPS * D], BF16, "uT2")
            nc.vector.tensor_copy(
                uT2[:, :].rearrange("q (k jl) -> q k jl", k=CAPS),
                vstack[:, :].rearrange("q (jl k) -> q k jl", k=CAPS),
            )
            nc.gpsimd.dma_start(
                out=uTd[:, :].rearrange("k (q jl) -> q k jl", q=PP // CAPS),
                in_=uT2[:, :].rearrange("q (k jl) -> q k jl", k=CAPS),
            )
            uT = sb.tile([CAPS, ROWS_PER_CORE], BF16, tag="uT")
            nc.sync.dma_start(out=uT[:, :], in_=uTd[:, :])

            # ---- projection: out[j,:] = uT[:,j].T @ lwg, bf16 staging ----
            # Groups of BCH row-chunks stage into one SBUF tile (copies all on
            # one engine, alternating per group); stores go out as one big
            # contiguous DMA per group via gpsimd, with a 1-elem absorber so
            # each store carries only its queue-slot wait.
            HM = M // 2
            BCH = 5
            for bt in range(JCH // BCH):
                osb = ob_pool.tile([128, BCH * M], BF16, tag=f"osb{bt % 2}")
                for ji in range(BCH):
                    jc = bt * BCH + ji
                    js = jc * 128
                    co = ji * M
                    psA = ps_pool.tile([128, HM], F32, tag="psA")
                    psB = ps_pool.tile([128, HM], F32, tag="psB")
                    nc.tensor.matmul(
                        psA[:, :], uT[:, js:js + 128], lwg_t[:, :HM],
                        start=True, stop=True,
                    )
                    nc.tensor.matmul(
                        psB[:, :], uT[:, js:js + 128], lwg_t[:, HM:],
                        start=True, stop=True,
                    )
                    if bt % 2 == 0:
                        nc.vector.tensor_copy(osb[:, co:co + HM], psA[:, :])
                        nc.vector.tensor_copy(osb[:, co + HM:co + M], psB[:, :])
                    else:
                        nc.scalar.copy(osb[:, co:co + HM], psA[:, :])
                        nc.scalar.copy(osb[:, co + HM:co + M], psB[:, :])
                r0 = bt * BCH * 128
                pab = ob_pool.tile([1, 1], BF16, tag=f"pabj{bt % 2}")
                nc.gpsimd.tensor_copy(pab[:, :], osb[0:1, BCH * M - 1:BCH * M])
                nc.gpsimd.dma_start(
                    out=outc[r0:r0 + BCH * 128, :].rearrange(
                        "(j p) m -> p j m", p=128
                    ),
                    in_=osb[:, :].rearrange("p (j m) -> p j m", j=BCH),
                )
    return nc


def _get_programs(A, ka):
    key = (A, ka)
    if key not in _BUILD_CACHE:
        nc1, nc2 = _build_phase1(ka), _build_phase2(A)
        _split_multiwait_waits(nc1)
        _split_multiwait_waits(nc2)
        _BUILD_CACHE[key] = (nc1, nc2)
    return _BUILD_CACHE[key]


def kernel(t, x, s, route_weights, larger_w, larger_b, elarger, tsv):
    t = int(t)
    x = np.ascontiguousarray(np.asarray(x, np.float32))
    tsv_t = np.asarray(tsv, np.float32)[t]
    allowed = np.nonzero(tsv_t != 0)[0]
    A = len(allowed)
    ka = (A + NC - 1) // NC

    nc1, nc2 = _get_programs(A, ka)

    # ---------- phase 1: priors, expert-parallel ----------
    rw = np.asarray(route_weights, np.float32)
    in1 = []
    for c in range(NC):
        xw_c = np.zeros((ka, INCH, B + ND), np.float32)
        for j in range(ka):
            g = c * ka + j
            if g < A:
                k = allowed[g]
                xw_c[j, :, :B] = x[:, k, :].T
                xw_c[j, :, B:] = rw[k].transpose(1, 0, 2).reshape(INCH, ND)
        in1.append({"xw": xw_c})
    res1 = run_bass_kernel_spmd(nc1, in1, list(range(NC)))
    LAST_RESULTS.append(res1)

    # priors_full[k, b, n, d]
    priors_full = np.zeros((A, B, CAPS, D), np.float32)
    for c in range(NC):
        pri = res1.results[c]["pri"]  # [ka, 2, 128, 600]
        for j in range(ka):
            g = c * ka + j
            if g < A:
                priors_full[g] = pri[j].reshape(B, CAPS, D)

    # ---------- phase 2: routing + projection, pair-parallel ----------
    g_gate = 1.0 / (1.0 + np.exp(-(np.float32(s[0]) * np.asarray(elarger, np.float32)[t])))
    lwg_f = (np.asarray(larger_w, np.float32) * g_gate[:, None]).T  # [3, 768]
    bg = np.asarray(larger_b, np.float32) * g_gate
    assert not np.any(bg), "nonzero larger_b not supported by this build"
    tsvA = tsv_t[allowed].astype(np.float32)

    in2 = []
    for c in range(NC):
        sidx = np.arange(c * PP, (c + 1) * PP)
        nv, bv = sidx // B, sidx % B
        P2 = priors_full[:, bv, nv, :].transpose(1, 0, 2)  # [96, A, 200]
        vs1 = P2.sum(axis=1) / A                      # uniform vote
        sq1 = (vs1 * vs1).sum(axis=1, keepdims=True)
        o1 = vs1 * (np.sqrt(sq1) / (1.0 + sq1))       # squash(vote)
        in2.append(
            {
                "pk": np.ascontiguousarray(P2.reshape(PP, A * D)),
                "pd": np.ascontiguousarray(
                    P2.transpose(0, 2, 1).reshape(PP, D * A)
                ).astype(np.float16),
                "o1in": np.ascontiguousarray(o1.astype(np.float32)),
                "tsvr": np.broadcast_to(tsvA, (PP, A)).copy(),
                "lwg": np.ascontiguousarray(lwg_f).astype(ml_dtypes.bfloat16),
            }
        )
    res2 = run_bass_kernel_spmd(nc2, in2, list(range(NC)))
    LAST_RESULTS.append(res2)

    out = np.concatenate(
        [res2.results[c]["outc"].astype(np.float32) for c in range(NC)], axis=0
    )
    return out.reshape(B, D, M)


# revision 33
# speedup vs baseline: 1.0193x; 1.0193x over previous
"""Trainium2 Bass kernel for nn_CapsuleLayerTSV (capsule routing over 40 adapters).

Strategy (8 NeuronCores, two SPMD NEFFs, no collectives):
  Phase 1 (expert-parallel): allowed adapters (tsv[t] != 0) sharded across
    cores, ka=3 slots each (pad slots zero-filled). Priors computed as
    float32r matmuls (12-bit-mantissa PE mode, 1 cyc/col) — [256,600]@[600,600]
    per adapter with capsules folded into the free dim.
  Host: reassemble priors, re-shard by the output's flat row space; ship each
    phase-2 core BOTH layouts of its priors (k-major for agreements, d-major
    for votes) so no on-chip transpose is needed.
  Phase 2 (pair-parallel): 3-iteration dynamic routing for 96 (n,b) pairs per
    core using big fused DVE ops (broadcast-multiply + 3D tensor_reduce over
    the innermost axis) instead of per-adapter op chains. Projection
    u[6400,3] @ lwg[3,768] in float32r; output staged to SBUF as bf16 and
    written as a bf16 DRAM tensor (host upcasts to f32).
"""

import sys

sys.path.insert(0, "/opt/trn_rl_repo")

import numpy as np
import ml_dtypes

import concourse.bass as bass
import concourse.mybir as mybir
import concourse.tile as tile
from concourse.bass import broadcast_tensor_aps
from concourse.bass_utils import run_bass_kernel_spmd

F32 = mybir.dt.float32
F32R = mybir.dt.float32r
BF16 = mybir.dt.bfloat16
F16 = mybir.dt.float16
AX = mybir.AxisListType
ALU = mybir.AluOpType
ACTF = mybir.ActivationFunctionType

NC = 8
B = 256
ADP = 40
CAPS = 3
INCH = 600
D = 200
M = 768
ND = CAPS * D  # 600
PP = CAPS * B // NC  # 96 (n,b) pairs per core in phase 2
ROWS_PER_CORE = PP * D // CAPS  # 6400 output rows per core
JCH = ROWS_PER_CORE // 128  # 50 row-chunks
KC = 5  # phase-1 contraction chunks of 120
KCS = INCH // KC  # 120

_BUILD_CACHE = {}
USE_GPSIMD_SPLIT = True


def _split_multiwait_waits(nc):
    """walrus caps sync-waits at ONE per instruction. For instructions executed
    by an in-order engine sequencer (everything except queue-executed DMAs),
    splitting the wait list across preceding 1-wait NoOps/Drains on the same
    engine is semantics-preserving."""
    for fn in nc.m.functions:
        for blk in fn.blocks:
            out = []
            for inst in blk.instructions:
                si = getattr(inst, "sync_info", None)
                if (
                    si is not None
                    and si.on_wait
                    and len(si.on_wait) > 1
                    and not isinstance(inst, mybir.InstDMACopy)
                    and getattr(inst, "engine", None) is not None
                ):
                    waits = list(si.on_wait)
                    cls = (
                        mybir.InstDrain
                        if isinstance(inst, mybir.InstDrain)
                        else mybir.InstNoOp
                    )
                    for i, w in enumerate(waits[:-1]):
                        extra = cls(
                            name=f"{inst.name}_w{i}",
                            engine=inst.engine,
                            sync_info=mybir.SyncInfo(on_wait=[w], on_update=[]),
                            bass_nofuse=True,
                        )
                        nc.register_instruction(extra)
                        out.append(extra)
                    si.on_wait = waits[-1:]
                out.append(inst)
            blk.instructions = out


# test/debug hook: kernel() appends the BassKernelResults of each phase here
LAST_RESULTS = []


def _build_phase1(ka):
    """SPMD program: priors for `ka` adapter slots per core, float32r.

    inputs : xw  [ka, 600, 856] f32   (cols 0:256 = x^T slice, 256:856 = W [c, n*d])
    output : pri [ka, 2, 128, 600] f32  (priors [b, n*d], b in 2 chunks of 128)
    """
    nc = bass.Bass()
    xw = nc.declare_dram_parameter("xw", [ka, INCH, B + ND], F32R, isOutput=False)
    pri = nc.declare_dram_parameter("pri", [ka, 2, 128, ND], F32, isOutput=True)

    with tile.TileContext(nc) as tc:
        with (
            tc.tile_pool(name="xt", bufs=3) as xt_pool,
            tc.tile_pool(name="ob", bufs=4) as ob_pool,
            tc.tile_pool(name="ps", bufs=2, space="PSUM") as ps_pool,
        ):
            for k in range(ka):
                # per-chunk HWDGE DMAs so matmuls start after the first chunk
                xwc = []
                for ci in range(KC):
                    cchunk = xt_pool.tile(
                        [KCS, B + ND], F32R, tag=f"xw{ci}", name=f"xw{k}_{ci}"
                    )
                    nc.sync.dma_start(
                        out=cchunk[:, :], in_=xw[k, ci * KCS:(ci + 1) * KCS, :]
                    )
                    xwc.append(cchunk)
                for bc in range(2):
                    for nh in range(2):
                        ps = ps_pool.tile([128, ND // 2], F32, tag=f"ps{bc}{nh}")
                        for ci in range(KC):
                            nc.tensor.matmul(
                                ps[:, :],
                                xwc[ci][:, bc * 128:(bc + 1) * 128],
                                xwc[ci][:, B + nh * 300:B + (nh + 1) * 300],
                                start=(ci == 0),
                                stop=(ci == KC - 1),
                            )
                        osb = ob_pool.tile([128, ND // 2], F32, tag=f"o{bc}{nh}")
                        dst = pri[k, bc, :, nh * 300:(nh + 1) * 300]
                        # copy psum->SBUF on vector/scalar; a 1-elem gpsimd
                        # absorber pulls the copy's sem into gpsimd's clock so
                        # the store DMA carries only its queue-slot wait
                        # (walrus allows ONE wait per DMA).
                        if nh == 0:
                            nc.vector.tensor_copy(osb[:, :], ps[:, :])
                        else:
                            nc.scalar.copy(osb[:, :], ps[:, :])
                        pab = ob_pool.tile([1, 1], F32, tag=f"pab{bc}{nh}")
                        nc.gpsimd.tensor_copy(pab[:, :], osb[0:1, 0:1])
                        nc.gpsimd.dma_start(out=dst, in_=osb[:, :])
    return nc


def _build_phase2(A):
    """SPMD program: routing for 96 (n,b) pairs + bf16 projection per core.

    inputs : pk   [96, A*200] f32  (k-major priors: [pair, k, d])
             pd   [96, 200*A] f16  (d-major priors: [pair, d, k], vote path)
             o1in [96, 200] f32    (squash of the uniform vote, host-computed)
             tsvr [96, A] f32      (tsv[t] over allowed adapters, replicated)
             lwg  [3, 768] bf16    (gate-folded projection, larger_b == 0)
    output : outc [6400, 768] bf16

    Algebra: the squashed output o_i never materializes for i>=2 —
    agreements use the unsquashed-but-normalized vote v (products stay in
    fp16 range) and the squash factor f = sqrt(sq)/(1+sq) scales the
    21-element agreement row afterwards: aT = f * sum_d(Pk * v).
    """
    nc = bass.Bass()
    pk = nc.declare_dram_parameter("pk", [PP, A * D], F32, isOutput=False)
    o1in = nc.declare_dram_parameter("o1in", [PP, D], F32, isOutput=False)
    pd = nc.declare_dram_parameter("pd", [PP, D * A], F16, isOutput=False)
    tsvr = nc.declare_dram_parameter("tsvr", [PP, A], F32, isOutput=False)
    lwg = nc.declare_dram_parameter("lwg", [CAPS, M], BF16, isOutput=False)
    outc = nc.declare_dram_parameter("outc", [ROWS_PER_CORE, M], BF16, isOutput=True)
    uTd = nc.dram_tensor("uTd", [CAPS, ROWS_PER_CORE], BF16)  # u^T staging

    KSPLITS = [0, 10, A]  # k-split points of the pk load/first multiply
    uid = [0]

    with tile.TileContext(nc) as tc:
        with (
            tc.tile_pool(name="sb", bufs=1) as sb,
            tc.tile_pool(name="ob", bufs=4) as ob_pool,
            tc.tile_pool(name="ut", bufs=1) as ut_pool,
            tc.tile_pool(name="ps", bufs=3, space="PSUM") as ps_pool,
        ):
            def fresh(shape, dtype=F32, pfx="t"):
                uid[0] += 1
                return sb.tile(
                    shape, dtype, tag=f"{pfx}{uid[0]}", name=f"{pfx}{uid[0]}"
                )

            # ---- loads (pk split in two so ag1 starts on the first half) ----
            Pk = sb.tile([PP, A * D], F32, tag="Pk")
            for a0, a1 in zip(KSPLITS[:-1], KSPLITS[1:]):
                nc.sync.dma_start(out=Pk[:, a0 * D:a1 * D], in_=pk[:, a0 * D:a1 * D])
            Pk3 = Pk[:, :].rearrange("p (k d) -> p k d", k=A)
            o1 = sb.tile([PP, D], F32, tag="o1")
            nc.sync.dma_start(out=o1[:, :], in_=o1in[:, :])
            Pd = sb.tile([PP, D * A], F16, tag="Pd")
            nc.sync.dma_start(out=Pd[:, :], in_=pd[:, :])
            Pd3 = Pd[:, :].rearrange("p (d k) -> p d k", d=D)
            tsv_t = sb.tile([PP, A], F32, tag="tsv")
            nc.sync.dma_start(out=tsv_t[:, :], in_=tsvr[:, :])
            lwg_t = sb.tile([CAPS, M], BF16, tag="lwg")
            nc.sync.dma_start(out=lwg_t[:, :], in_=lwg[:, :])

            scrK = sb.tile([PP, A * D], F16, tag="scrK")
            scrK3 = scrK[:, :].rearrange("p (k d) -> p k d", k=A)
            scrD = sb.tile([PP, D * A], F16, tag="scrD")
            scrD3 = scrD[:, :].rearrange("p (d k) -> p d k", d=D)

            psj = ps_pool.tile([128, 512], F32, tag="psj", bufs=1)

            def pe_warm():
                nc.tensor.matmul(
                    psj[:, :], lwg_t[:, 0:128], lwg_t[:, 0:512],
                    start=True, stop=True,
                )

            def agree_raw(v_t, aT, split=False):
                """aT[p,k] = sum_d Pk[p,k,d] * v_t[p,d]  (mult + X-reduce)."""
                v3 = v_t[:, :].rearrange("p (u d) -> p u d", u=1)
                v3b, Pk3b = broadcast_tensor_aps(v3, Pk3)
                if split:
                    for a0, a1 in zip(KSPLITS[:-1], KSPLITS[1:]):
                        nc.vector.tensor_tensor(
                            out=scrK3[:, a0:a1, :], in0=Pk3b[:, a0:a1, :],
                            in1=v3b[:, a0:a1, :], op=ALU.mult,
                        )
                else:
                    nc.vector.tensor_tensor(
                        out=scrK3, in0=Pk3b, in1=v3b, op=ALU.mult
                    )
                nc.vector.tensor_reduce(aT[:, :], scrK3, AX.X, ALU.add)

            def vote_weighted(e_t, vs):
                """vs[p,d] = sum_k e_t[p,k] * Pd[p,d,k]  (all-fp16 mult + X-reduce)."""
                e3 = e_t[:, :].rearrange("p (u k) -> p u k", u=1)
                e3b, Pd3b = broadcast_tensor_aps(e3, Pd3)
                nc.vector.tensor_tensor(out=scrD3, in0=Pd3b, in1=e3b, op=ALU.mult)
                nc.vector.tensor_reduce(vs[:, :], scrD3, AX.X, ALU.add)

            def softmax_from_logit(logit):
                """returns (e fp16, dinv f32): e = exp(logit - max); dinv = 1/sum."""
                rmax = fresh([PP, 1], F32, "rmx")
                am = fresh([PP, A], F32, "am")
                e = fresh([PP, A], F16, "e")
                dsum = fresh([PP, 1], F32, "dsm")
                dinv = fresh([PP, 1], F32, "dnv")
                nc.vector.tensor_reduce(rmax[:, :], logit[:, :], AX.X, ALU.max)
                nc.vector.tensor_scalar(
                    out=am[:, :], in0=logit[:, :], scalar1=rmax[:, 0:1],
                    scalar2=None, op0=ALU.subtract,
                )
                nc.scalar.activation(
                    e[:, :], am[:, :], ACTF.Exp, accum_out=dsum[:, 0:1]
                )
                nc.vector.reciprocal(dinv[:, :], dsum[:, :])
                return e, dinv

            def squash_scale(v_t):
                """f = sqrt(sq)/(1+sq), sq = sum(v_t^2); sqrt via DVE pow."""
                junk = fresh([PP, D], F32, "sqj")
                sq = fresh([PP, 1], F32, "sq")
                sqs = fresh([PP, 1], F32, "sqs")
                sp1 = fresh([PP, 1], F32, "sp1")
                rec = fresh([PP, 1], F32, "rec")
                f = fresh([PP, 1], F32, "f")
                nc.vector.scalar_tensor_tensor(
                    out=junk[:, :], in0=v_t[:, :], scalar=1.0, in1=v_t[:, :],
                    op0=ALU.mult, op1=ALU.mult, accum_out=sq[:, 0:1],
                )
                nc.scalar.sqrt(sqs[:, :], sq[:, :])
                nc.vector.tensor_scalar(
                    out=sp1[:, :], in0=sq[:, :], scalar1=1.0, scalar2=None,
                    op0=ALU.add,
                )
                nc.vector.reciprocal(rec[:, :], sp1[:, :])
                nc.vector.tensor_tensor(
                    out=f[:, :], in0=sqs[:, :], in1=rec[:, :], op=ALU.mult
                )
                return f

            def iteration(e_t, dinv, lg_prev, last):
                """One routing iteration from softmax weights; returns next
                logits (or the final normalized vote when last=True)."""
                vs = fresh([PP, D], F32, "vs")
                vote_weighted(e_t, vs)
                v = fresh([PP, D], F32, "v")
                nc.vector.tensor_scalar(
                    out=v[:, :], in0=vs[:, :], scalar1=dinv[:, 0:1],
                    scalar2=None, op0=ALU.mult,
                )
                if last:
                    return v
                f = squash_scale(v)
                agp = fresh([PP, A], F32, "agp")
                agree_raw(v, agp)
                aT = fresh([PP, A], F32, "aT")
                nc.vector.tensor_scalar(
                    out=aT[:, :], in0=agp[:, :], scalar1=f[:, 0:1],
                    scalar2=None, op0=ALU.mult,
                )
                lg = fresh([PP, A], F32, "lg")
                nc.vector.scalar_tensor_tensor(
                    out=lg[:, :], in0=lg_prev[:, :], scalar=1.0, in1=aT[:, :],
                    op0=ALU.mult, op1=ALU.add,
                )
                nc.vector.tensor_tensor(
                    out=lg[:, :], in0=lg[:, :], in1=tsv_t[:, :], op=ALU.mult
                )
                return lg

            # ---- iteration 1: uniform probs; o1 shipped from host ----
            aT1 = fresh([PP, A], F32, "aT1")
            agree_raw(o1, aT1, split=True)
            lg1 = fresh([PP, A], F32, "lg1")
            nc.vector.tensor_tensor(
                out=lg1[:, :], in0=aT1[:, :], in1=tsv_t[:, :], op=ALU.mult
            )
            pe_warm()

            # ---- iteration 2 ----
            e2, dinv2 = softmax_from_logit(lg1)
            pe_warm()
            lg2 = iteration(e2, dinv2, lg1, last=False)
            pe_warm()

            # ---- iteration 3: final vote ----
            e3, dinv3 = softmax_from_logit(lg2)
            pe_warm()
            vs3 = fresh([PP, D], F32, "vs3")
            vote_weighted(e3, vs3)
            vb16 = ut_pool.tile([PP, D], BF16, tag="vb16")
            nc.vector.tensor_scalar(
                out=vb16[:, :], in0=vs3[:, :], scalar1=dinv3[:, 0:1],
                scalar2=None, op0=ALU.mult,
            )
            pe_warm()

            # ---- deinterleave vote stream into u^T rows (via DRAM) ----
            vstack = ut_pool.tile([PP // CAPS, CAPS * D], BF16, tag="vstk")
            nc.gpsimd.dma_start(
                out=vstack[:, :].rearrange("q (m d) -> q m d", m=CAPS),
                in_=vb16[:, :],
            )
            uT2 = ut_pool.tile([PP // CAPS, CAPS * D], BF16, tag="uT2")
            nc.vector.tensor_copy(
                uT2[:, :].rearrange("q (k jl) -> q k jl", k=CAPS),
                vstack[:, :].rearrange("q (jl k) -> q k jl", k=CAPS),
            )
            nc.gpsimd.dma_start(
                out=uTd[:, :].rearrange("k (q jl) -> q k jl", q=PP // CAPS),
                in_=uT2[:, :].rearrange("q (k jl) -> q k jl", k=CAPS),
            )
            uT = ut_pool.tile([CAPS, ROWS_PER_CORE], BF16, tag="uT")
            nc.sync.dma_start(out=uT[:, :], in_=uTd[:, :])

            # ---- projection: out[j,:] = uT[:,j].T @ lwg, bf16 staging ----
            # Groups of BCH row-chunks stage into one SBUF tile (copies all on
            # one engine, alternating per group); stores go out as one big
            # contiguous DMA per group via gpsimd, with a 1-elem absorber so
            # each store carries only its queue-slot wait.
            HM = M // 2
            BCH = 5
            for bt in range(JCH // BCH):
                osb = ob_pool.tile([128, BCH * M], BF16, tag=f"osb{bt % 2}")
                for ji in range(BCH):
                    jc = bt * BCH + ji
                    js = jc * 128
                    co = ji * M
                    psA = ps_pool.tile([128, HM], F32, tag="psA")
                    psB = ps_pool.tile([128, HM], F32, tag="psB")
                    nc.tensor.matmul(
                        psA[:, :], uT[:, js:js + 128], lwg_t[:, :HM],
                        start=True, stop=True,
                    )
                    nc.tensor.matmul(
                        psB[:, :], uT[:, js:js + 128], lwg_t[:, HM:],
                        start=True, stop=True,
                    )
                    if bt % 2 == 0:
                        nc.vector.tensor_copy(osb[:, co:co + HM], psA[:, :])
                        nc.vector.tensor_copy(osb[:, co + HM:co + M], psB[:, :])
                    else:
                        nc.scalar.copy(osb[:, co:co + HM], psA[:, :])
                        nc.scalar.copy(osb[:, co + HM:co + M], psB[:, :])
                r0 = bt * BCH * 128
                pab = ob_pool.tile([1, 1], BF16, tag=f"pabj{bt % 2}")
                nc.gpsimd.tensor_copy(pab[:, :], osb[0:1, BCH * M - 1:BCH * M])
                nc.gpsimd.dma_start(
                    out=outc[r0:r0 + BCH * 128, :].rearrange(
                        "(j p) m -> p j m", p=128
                    ),
                    in_=osb[:, :].rearrange("p (j m) -> p j m", j=BCH),
                )
    return nc


def _get_programs(A, ka):
    key = (A, ka)
    if key not in _BUILD_CACHE:
        nc1, nc2 = _build_phase1(ka), _build_phase2(A)
        _split_multiwait_waits(nc1)
        _split_multiwait_waits(nc2)
        _BUILD_CACHE[key] = (nc1, nc2)
    return _BUILD_CACHE[key]


def kernel(t, x, s, route_weights, larger_w, larger_b, elarger, tsv):
    t = int(t)
    x = np.ascontiguousarray(np.asarray(x, np.float32))
    tsv_t = np.asarray(tsv, np.float32)[t]
    allowed = np.nonzero(tsv_t != 0)[0]
    A = len(allowed)
    ka = (A + NC - 1) // NC

    nc1, nc2 = _get_programs(A, ka)

    # ---------- phase 1: priors, expert-parallel ----------
    rw = np.asarray(route_weights, np.float32)
    in1 = []
    for c in range(NC):
        xw_c = np.zeros((ka, INCH, B + ND), np.float32)
        for j in range(ka):
            g = c * ka + j
            if g < A:
                k = allowed[g]
                xw_c[j, :, :B] = x[:, k, :].T
                xw_c[j, :, B:] = rw[k].transpose(1, 0, 2).reshape(INCH, ND)
        in1.append({"xw": xw_c})
    res1 = run_bass_kernel_spmd(nc1, in1, list(range(NC)))
    LAST_RESULTS.append(res1)

    # priors_full[k, b, n, d]
    priors_full = np.zeros((A, B, CAPS, D), np.float32)
    for c in range(NC):
        pri = res1.results[c]["pri"]  # [ka, 2, 128, 600]
        for j in range(ka):
            g = c * ka + j
            if g < A:
                priors_full[g] = pri[j].reshape(B, CAPS, D)

    # ---------- phase 2: routing + projection, pair-parallel ----------
    g_gate = 1.0 / (1.0 + np.exp(-(np.float32(s[0]) * np.asarray(elarger, np.float32)[t])))
    lwg_f = (np.asarray(larger_w, np.float32) * g_gate[:, None]).T  # [3, 768]
    bg = np.asarray(larger_b, np.float32) * g_gate
    assert not np.any(bg), "nonzero larger_b not supported by this build"
    tsvA = tsv_t[allowed].astype(np.float32)

    in2 = []
    for c in range(NC):
        sidx = np.arange(c * PP, (c + 1) * PP)
        nv, bv = sidx // B, sidx % B
        P2 = priors_full[:, bv, nv, :].transpose(1, 0, 2)  # [96, A, 200]
        vs1 = P2.sum(axis=1) / A                      # uniform vote
        sq1 = (vs1 * vs1).sum(axis=1, keepdims=True)
        o1 = vs1 * (np.sqrt(sq1) / (1.0 + sq1))       # squash(vote)
        in2.append(
            {
                "pk": np.ascontiguousarray(P2.reshape(PP, A * D)),
                "pd": np.ascontiguousarray(
                    P2.transpose(0, 2, 1).reshape(PP, D * A)
                ).astype(np.float16),
                "o1in": np.ascontiguousarray(o1.astype(np.float32)),
                "tsvr": np.broadcast_to(tsvA, (PP, A)).copy(),
                "lwg": np.ascontiguousarray(lwg_f).astype(ml_dtypes.bfloat16),
            }
        )
    res2 = run_bass_kernel_spmd(nc2, in2, list(range(NC)))
    LAST_RESULTS.append(res2)

    out = np.concatenate(
        [res2.results[c]["outc"].astype(np.float32) for c in range(NC)], axis=0
    )
    return out.reshape(B, D, M)
